# revision 21
# baseline (speedup 1.0000x reference)
"""GAT message-passing kernel for Trainium2 (8 NeuronCores, data-parallel over batch).

Math (per batch element b, derived from the reference nn.Module):
    x      = nodes.reshape(N, D)
    self_e = mlp2(x, self_*)                 # [N, H]
    nb_e   = mlp2(x, nb_*)                   # [N, H]
    U      = self_e @ comb_w1[:H]            # [N, H]  (i side)
    V      = nb_e @ comb_w1[H:] + comb_b1    # [N, H]  (j side)
    scores(i,j) = leaky(U_i + V_j) @ w2 + b2
                = 0.8*relu(U_i+V_j)@w2 + 0.2*(sU_i + sV_j) + const_i
    Softmax over j is invariant to per-i constants, so only
      s'(i,j) = 0.8*relu(U_i+V_j)@w2 + 0.2*sV_j  matters, and
      exp(s') factorizes as exp(0.8 relu(...)@w2) * exp(0.2 sV_j).
    ET[j,i]  = edges[j,i]*(j!=i) * exp(0.8 relu(U_i+V_j)@w2)
    den[i]   = sum_j ET[j,i]*esv_j      (esv_j = exp(0.2 sV_j))
    agg[i,:] = sum_j ET[j,i]*esv_j*nb_e[j,:]
    out[i]   = (den>eps) * (agg/den + self_e)
    (|scores| < 2, so exp needs no max-subtraction.)

Device mapping (one core per batch element):
  - Transposed (g,h)-on-partitions layout: partitions = (i-parity g, h), so one
    tensor_scalar(add,max) / activation(Relu,bias) op builds relu(V + U_i) for
    TWO i's at once as a [128, 512] tile.
  - PE reduces over (g,h) with slotted block-diagonal 0.8*w2 lhsT. Most pairs
    of slots go through ONE fp8 MatmulPerfMode.DoubleRow matmul (two K-planes,
    0.5 cycles/row -> 107 ns for 4 i's); DVE-built slots may instead use bf16
    single-slot matmuls (cheaper on DVE: 4x perf mode needs 2-byte dtypes).
    The per-pair engine/dtype assignment is tunable (GAT_PAIRS).
  - ACT applies exp straight out of PSUM (bf16); PE transposes 128x128 chunks;
    DVE/Pool multiply by mask tiles (edges * (1-eye), loaded via gpsimd
    cast-DMA u8->bf16) -> ET.
  - The per-j factor exp(0.2 sV_j) and the softmax denominator are folded into
    the aggregation matmul: rhs = [esv*nb_e | esv] (65 cols), so den arrives
    as PSUM column 64 already per-partition -- no row->column scatter.
  - Precompute MLP matmuls run as float32r (1 cycle/row at >=256 free dim,
    exact fp32 in sim); self_e / nb_e come from PE transposes of the already
    biased eT_s / eT_n (no extra matmuls).
  - fp8e4m3 quantization of the relu tiles + 0.8*w2 keeps absmax rel err
    ~9e-4 (measured offline vs fp64 reference).
"""

import os
import sys

sys.path.insert(0, "/opt/trn_rl_repo")

import numpy as np
import ml_dtypes

import concourse.bass as bass
import concourse.bacc as bacc
import concourse.tile as tile
from concourse import mybir, bass2jax
from concourse.bass_utils import run_bass_kernel_spmd

B, N, H, D = 8, 512, 64, 128
NCORES = 8
NT = N // 128          # 4 i/j tiles of 128
NPAIR = N // 2         # 256 i-pairs
F32 = mybir.dt.float32
F32R = mybir.dt.float32r
BF16 = mybir.dt.bfloat16
FP8 = mybir.dt.float8e4
I32 = mybir.dt.int32

# Per slot-pair engine assignment for the 128 pairs (4 it x 2 c x 16 t):
#   'b' = two bf16 builds on DVE + two bf16 single-slot matmuls
#   'v'/'a'/'p' = two fp8 builds on DVE/ACT/Pool + one fp8 DoubleRow matmul
# Either a 128-char string or comma counts like "b57,v12,a24,p35".
PAIR_SPEC = os.environ.get("GAT_PAIRS", "b55,v13,a25,p35")
# Engine for the 16 ET mask-multiplies (4 it x 4 jt): 'v' = DVE, 'p' = Pool.
ET_SPEC = os.environ.get("GAT_ETPAT", "v" * 16)

_CACHE = {}


def _expand_pairs(spec):
    if "," not in spec and len(spec) == 128:
        return spec
    counts = {}
    for part in spec.split(","):
        counts[part[0]] = int(part[1:])
    assert sum(counts.values()) == 128, counts
    # Bresenham-style proportional interleave for even engine spacing
    acc = {k: 0.0 for k in counts}
    out = []
    for _ in range(128):
        for k in acc:
            acc[k] += counts[k] / 128.0
        best = max(acc, key=lambda k: acc[k])
        acc[best] -= 1.0
        out.append(best)
    return "".join(out)


def _build_module():
    nc = bacc.Bacc("TRN2", target_bir_lowering=False, debug=False, num_devices=NCORES)

    # ---- per-core data ----
    nodes = nc.dram_tensor("nodes", [N, D], F32, kind="ExternalInput")
    edges = nc.dram_tensor("edges", [N, N], mybir.dt.uint8, kind="ExternalInput")
    # ---- parameters / host-prepared constants (same on all cores) ----
    w1_self = nc.dram_tensor("w1_self", [D, H], F32, kind="ExternalInput")
    w2_self = nc.dram_tensor("w2_self", [H, H], F32, kind="ExternalInput")
    w1_nb = nc.dram_tensor("w1_nb", [D, H], F32, kind="ExternalInput")
    w2_nb = nc.dram_tensor("w2_nb", [H, H], F32, kind="ExternalInput")
    w1_cs = nc.dram_tensor("w1_cs", [H, H], F32, kind="ExternalInput")
    w1_cn = nc.dram_tensor("w1_cn", [H, H], F32, kind="ExternalInput")
    w2_c = nc.dram_tensor("w2_c", [H, 1], BF16, kind="ExternalInput")
    bvec = nc.dram_tensor("bvec", [H, 5], F32, kind="ExternalInput")
    w2bdpack = nc.dram_tensor("w2bdpack", [128, 2], F32, kind="ExternalInput")
    id_f32 = nc.dram_tensor("id_f32", [128, 128], F32, kind="ExternalInput")
    inveye = nc.dram_tensor("inveye", [128, 128], BF16, kind="ExternalInput")

    out = nc.dram_tensor("out", [N, H], F32, kind="ExternalOutput")

    with tile.TileContext(nc) as tc:
        _emit(nc, tc, locals())
    nc.compile()
    return nc


def _emit(nc, tc, t):
    AF = mybir.ActivationFunctionType
    OP = mybir.AluOpType
    pairs = _expand_pairs(PAIR_SPEC)

    with (
        tc.tile_pool(name="persist", bufs=1) as P,
        tc.tile_pool(name="xwork", bufs=2) as XW,
        tc.tile_pool(name="ework", bufs=2) as EW,
        tc.tile_pool(name="relb", bufs=14) as RLB,
        tc.tile_pool(name="rel8", bufs=24) as RL8,
        tc.tile_pool(name="xexp", bufs=2) as XE,
        tc.tile_pool(name="xtr", bufs=4) as PXS,
        tc.tile_pool(name="etw", bufs=4) as ETW,
        tc.tile_pool(name="small", bufs=4) as SM,
        tc.tile_pool(name="psumR", bufs=2, space="PSUM") as PR,
        tc.tile_pool(name="psumT", bufs=3, space="PSUM") as PT,
        tc.tile_pool(name="psumM", bufs=1, space="PSUM") as PM,
        tc.tile_pool(name="psumC", bufs=1, space="PSUM") as PC,
        tc.tile_pool(name="psumA", bufs=1, space="PSUM") as PA,
    ):
        # ---------- load constants ----------
        def load(name, shape, dtype, eng=None):
            tl = P.tile(shape, dtype, tag=name)
            (eng or nc.sync).dma_start(out=tl[:], in_=t[name].ap())
            return tl

        xins = []
        for it in range(NT):
            xin = XW.tile([128, D], F32, name="xin", tag="xin")
            nc.sync.dma_start(out=xin[:], in_=t["nodes"].ap()[bass.ts(it, 128), :])
            xins.append(xin)
        w1s = load("w1_self", [D, H], F32)
        w1n = load("w1_nb", [D, H], F32)
        w2s = load("w2_self", [H, H], F32)
        w2n = load("w2_nb", [H, H], F32)
        w1cs = load("w1_cs", [H, H], F32)
        w1cn = load("w1_cn", [H, H], F32)
        w2cb = load("w2_c", [H, 1], BF16)
        bvec = load("bvec", [H, 5], F32)
        b1s, b1n = bvec[:, 0:1], bvec[:, 1:2]
        b2sc, b2nc, b1c = bvec[:, 2:3], bvec[:, 3:4], bvec[:, 4:5]
        idf = P.tile([128, 128], F32, tag="id_f32")
        nc.sync.dma_start(out=idf[:], in_=t["id_f32"].ap())
        ive = P.tile([128, 128], BF16, tag="ive")
        nc.sync.dma_start(out=ive[:], in_=t["inveye"].ap())
        w2bdf = P.tile([128, 2], F32, tag="w2bdf")
        nc.sync.dma_start(out=w2bdf[:], in_=t["w2bdpack"].ap())
        # bf16 block-diagonal buffer for single-slot matmuls
        w2bd_all = P.tile([128, 128], BF16, tag="w2bd_all")
        nc.gpsimd.memset(w2bd_all[:], 0.0)
        nc.vector.tensor_copy(out=w2bd_all[:, 62:64], in_=w2bdf[:])
        w2bd_sb = [w2bd_all[:, 62 - 2 * s:126 - 2 * s] for s in range(32)]
        # fp8 DoubleRow weights: plane 0 = window buf A (cols 62:64 hot),
        # plane 1 = window buf B (cols 64:66 hot); free layout [2, 130]
        w2dr = P.tile([128, 2, 130], FP8, tag="w2dr")
        nc.gpsimd.memset(w2dr[:], 0.0)
        nc.vector.tensor_copy(out=w2dr[:, 0, 62:64], in_=w2bdf[:])
        nc.vector.tensor_copy(out=w2dr[:, 1, 64:66], in_=w2bdf[:])

        # ---------- masks: edges * (1 - eye), bf16 via casting gpsimd DMA ----
        masks = []
        for jt in range(NT):
            mj = P.tile([128, N], BF16, tag=f"mask{jt}", name=f"mask{jt}")
            nc.gpsimd.dma_start(out=mj[:], in_=t["edges"].ap()[bass.ts(jt, 128), :])
            nc.vector.tensor_mul(out=mj[:, bass.ts(jt, 128)], in0=mj[:, bass.ts(jt, 128)],
                                 in1=ive[:])
            masks.append(mj)

        # ---------- x -> x^T ----------
        xT = P.tile([D, N], F32, tag="xT")
        for it in range(NT):
            px = PT.tile([128, 128], F32, tag="pt", name="px", padded_shape=[128, 128])
            nc.tensor.transpose(px[:], xins[it][:], idf[:])
            nc.vector.tensor_copy(out=xT[:, bass.ts(it, 128)], in_=px[:])
        xTr = xT.bitcast(F32R)

        # ---------- tiny MLPs (transposed; h on partitions), f32r matmuls ----
        pm = PM.tile([128, N], F32, tag="mm", name="pm_n1")
        nc.tensor.matmul(pm[:H, :], w1n.bitcast(F32R)[:], xTr[:], start=True, stop=True)
        z = EW.tile([H, N], F32, tag="lk_z")
        nc.gpsimd.tensor_scalar_add(out=z[:], in0=pm[:H, :], scalar1=b1n)
        h1T_n = P.tile([H, N], F32, tag="h1T_n")
        nc.vector.scalar_tensor_tensor(out=h1T_n[:], in0=z[:], scalar=0.2,
                                       in1=z[:], op0=OP.mult, op1=OP.max)

        pm = PM.tile([128, N], F32, tag="mm", name="pm_n2")
        nc.tensor.matmul(pm[:H, :], w2n.bitcast(F32R)[:], h1T_n.bitcast(F32R)[:],
                         start=True, stop=True)
        eT_n = P.tile([H, N], F32, tag="eT_n")
        nc.gpsimd.tensor_scalar_add(out=eT_n[:], in0=pm[:H, :], scalar1=b2nc)

        # Vrep (bf16, both partition halves) straight from PSUM
        pm = PM.tile([128, N], F32, tag="mm", name="pm_n3")
        nc.tensor.matmul(pm[:H, :], w1cn.bitcast(F32R)[:], eT_n.bitcast(F32R)[:],
                         start=True, stop=True)
        Vrep = P.tile([128, N], BF16, tag="Vrep")
        nc.scalar.activation(out=Vrep[:H, :], in_=pm[:H, :], func=AF.Identity,
                             bias=b1c, scale=1.0)
        nc.vector.tensor_scalar_add(out=Vrep[H:, :], in0=pm[:H, :], scalar1=b1c)

        # self chain, two 256-column chunks so U2's early columns land early
        h1T_s = P.tile([H, N], F32, tag="h1T_s")
        eT_s = P.tile([H, N], F32, tag="eT_s")
        U2 = P.tile([128, NPAIR], F32, tag="U2")
        for ch in range(2):
            cs = bass.ts(ch, 256)
            pc = PC.tile([128, 256], F32, tag="pc", name="pc1")
            nc.tensor.matmul(pc[:H, :], w1s.bitcast(F32R)[:], xTr[:, cs],
                             start=True, stop=True)
            zc = EW.tile([H, 256], F32, tag="lk_zc", name="zc")
            nc.gpsimd.tensor_scalar_add(out=zc[:], in0=pc[:H, :], scalar1=b1s)
            nc.vector.scalar_tensor_tensor(out=h1T_s[:, cs], in0=zc[:], scalar=0.2,
                                           in1=zc[:], op0=OP.mult, op1=OP.max)
            pc = PC.tile([128, 256], F32, tag="pc", name="pc2")
            nc.tensor.matmul(pc[:H, :], w2s.bitcast(F32R)[:], h1T_s.bitcast(F32R)[:, cs],
                             start=True, stop=True)
            nc.gpsimd.tensor_scalar_add(out=eT_s[:, cs], in0=pc[:H, :], scalar1=b2sc)
            pc = PC.tile([128, 256], F32, tag="pc", name="pc3")
            nc.tensor.matmul(pc[:H, :], w1cs.bitcast(F32R)[:], eT_s.bitcast(F32R)[:, cs],
                             start=True, stop=True)
            psplit = pc[:H, :].rearrange("p (i g) -> p i g", g=2)
            nc.gpsimd.tensor_scalar_add(out=U2[:H, bass.ts(ch, 128)],
                                        in0=psplit[:, :, 0], scalar1=0.0)
            nc.gpsimd.tensor_scalar_add(out=U2[H:, bass.ts(ch, 128)],
                                        in0=psplit[:, :, 1], scalar1=0.0)

        # exp(0.2 * sV) row -> [128, NT] per-partition scalars
        pm = PM.tile([128, N], F32, tag="mm", name="pm_sv")
        nc.tensor.matmul(pm[:1, :], w2cb[:], Vrep[:H, :], start=True, stop=True)
        sv_row = SM.tile([1, N], F32, tag="sv_row")
        nc.scalar.activation(out=sv_row[:], in_=pm[:1, :], func=AF.Exp, scale=0.2)
        pesv = PT.tile([128, 128], F32, tag="pt", name="pesv", padded_shape=[128, 128])
        for tq in range(NT):
            nc.tensor.transpose(pesv[:, tq:tq + 1], sv_row[:, bass.ts(tq, 128)],
                                idf[0:1, 0:1])
        esv = P.tile([128, NT], F32, tag="esv")
        nc.vector.tensor_copy(out=esv[:], in_=pesv[:, 0:NT])

        # ---------- self_e [i,H] via transpose of eT_s; nbe2 = [esv*nb_e|esv] --
        selfe = []
        for it in range(NT):
            pT = PT.tile([128, 128], F32, tag="pt", name="pTs", padded_shape=[128, 128])
            nc.tensor.transpose(pT[:, 0:64], eT_s[:, bass.ts(it, 128)], idf[0:64, 0:64])
            se = P.tile([128, H], F32, tag=f"selfe{it}")
            nc.gpsimd.tensor_scalar_add(out=se[:], in0=pT[:, 0:64], scalar1=0.0)
            selfe.append(se)
        nbe2 = []
        for jt in range(NT):
            pT = PT.tile([128, 128], F32, tag="pt", name="pTn", padded_shape=[128, 128])
            nc.tensor.transpose(pT[:, 0:64], eT_n[:, bass.ts(jt, 128)], idf[0:64, 0:64])
            ne = P.tile([128, H + 1], BF16, tag=f"nbe{jt}")
            nc.gpsimd.tensor_scalar_mul(out=ne[:, 0:H], in0=pT[:, 0:64],
                                        scalar1=esv[:, jt:jt + 1])
            nc.gpsimd.tensor_copy(out=ne[:, H:H + 1], in_=esv[:, jt:jt + 1])
            nbe2.append(ne)

        # ---------- main pass: scores -> exp -> ET -> agg+den -> out ----------
        def emit_build(eng, out_ap, p):
            u = U2[:, p:p + 1]
            if eng == "v" or eng == "b":
                nc.vector.tensor_scalar(out=out_ap, in0=Vrep[:], scalar1=u,
                                        scalar2=0.0, op0=OP.add, op1=OP.max)
            elif eng == "a":
                nc.scalar.activation(out=out_ap, in_=Vrep[:], func=AF.Relu,
                                     bias=u, scale=1.0)
            else:
                nc.gpsimd.tensor_scalar(out=out_ap, in0=Vrep[:], scalar1=u,
                                        scalar2=0.0, op0=OP.add, op1=OP.max)

        def emit_group(it, c, ps):
            glist = [pairs[(it * 2 + c) * 16 + tt] for tt in range(16)]
            if it == NT - 1 and c == 1:
                # drain the slow build engines first so the tail is short
                order = sorted(range(16), key=lambda tt: "apvb".index(glist[tt]))
            else:
                order = list(range(16))
            first, last = order[0], order[-1]
            for tt in order:
                eng = glist[tt]
                p0 = 64 * it + 32 * c + 2 * tt
                if eng == "b":
                    for g in range(2):
                        rl = RLB.tile([128, N], BF16, tag="rlb")
                        emit_build("b", rl[:], p0 + g)
                        nc.tensor.matmul(ps[bass.ts(c, 64), :], w2bd_sb[2 * tt + g],
                                         rl[:], start=(tt == first and g == 0),
                                         stop=(tt == last and g == 1))
                else:
                    rl2 = RL8.tile([128, 2, N], FP8, tag="rl8")
                    emit_build(eng, rl2[:, 0, :], p0)
                    emit_build(eng, rl2[:, 1, :], p0 + 1)
                    nc.tensor.matmul(ps[bass.ts(c, 64), :],
                                     w2dr[:, :, 62 - 4 * tt:126 - 4 * tt], rl2[:],
                                     start=(tt == first), stop=(tt == last),
                                     perf_mode=mybir.MatmulPerfMode.DoubleRow)

        def emit_post(it, ps):
            X = XE.tile([128, N], BF16, tag="X")
            nc.scalar.activation(out=X[:], in_=ps[:], func=AF.Exp)
            pa = PA.tile([128, H + 1], F32, tag="pa", name="pa")
            for jt in range(NT):
                px = PXS.tile([128, 128], BF16, tag="pxs")
                nc.sync.dma_start_transpose(out=px[:], in_=X[:, bass.ts(jt, 128)])
                etw = ETW.tile([128, 128], BF16, tag="etw")
                if ET_SPEC[it * NT + jt] == "p":
                    nc.gpsimd.tensor_mul(out=etw[:], in0=px[:],
                                         in1=masks[jt][:, bass.ts(it, 128)])
                else:
                    nc.vector.tensor_mul(out=etw[:], in0=px[:],
                                         in1=masks[jt][:, bass.ts(it, 128)])
                nc.tensor.matmul(pa[:], etw[:], nbe2[jt][:],
                                 start=(jt == 0), stop=(jt == NT - 1))
            den = pa[:, H:H + 1]
            # no-neighbor rows have agg == 0 exactly, so an ungated reciprocal
            # (1e30) still yields 0 for the agg term; only selfe needs the gate
            asm = nc.vector if it == NT - 1 else nc.gpsimd
            gate = SM.tile([128, 1], F32, tag="gate", name="gate")
            asm.tensor_single_scalar(out=gate[:], in_=den, scalar=1e-6, op=OP.is_gt)
            dsafe = SM.tile([128, 1], F32, tag="dsafe", name="dsafe")
            asm.tensor_scalar_max(out=dsafe[:], in0=den, scalar1=1e-30)
            recip = SM.tile([128, 1], F32, tag="recip", name="recip")
            nc.vector.reciprocal(out=recip[:], in_=dsafe[:])
            sg = SM.tile([128, H], F32, tag="sg")
            nc.vector.tensor_scalar_mul(out=sg[:], in0=selfe[it][:], scalar1=gate[:])
            ot = SM.tile([128, H], F32, tag="ot")
            nc.vector.scalar_tensor_tensor(out=ot[:], in0=pa[:, 0:H],
                                           scalar=recip[:], in1=sg[:],
                                           op0=OP.mult, op1=OP.add)
            nc.sync.dma_start(out=t["out"].ap()[bass.ts(it, 128), :], in_=ot[:])

        ps_tiles = [None] * NT
        for it in range(NT):
            ps = PR.tile([128, N], F32, tag="psumR", name=f"ps{it}")
            ps_tiles[it] = ps
            emit_group(it, 0, ps)
            if it > 0:
                emit_post(it - 1, ps_tiles[it - 1])
            emit_group(it, 1, ps)
        emit_post(NT - 1, ps_tiles[NT - 1])


def _host_constants(inputs):
    f32 = np.float32
    bf = ml_dtypes.bfloat16
    H_ = H
    w2 = np.asarray(inputs["comb_w2"], f32)            # [H, 1]
    w2bdpack = np.zeros((128, 2), f32)
    w2bdpack[0:H_, 0] = 0.8 * w2[:, 0]
    w2bdpack[H_:128, 1] = 0.8 * w2[:, 0]
    ive = (1.0 - np.eye(128)).astype(f32)
    consts = {
        "w1_self": np.asarray(inputs["self_w1"], f32),
        "w2_self": np.asarray(inputs["self_w2"], f32),
        "w1_nb": np.asarray(inputs["nb_w1"], f32),
        "w2_nb": np.asarray(inputs["nb_w2"], f32),
        "w1_cs": np.ascontiguousarray(np.asarray(inputs["comb_w1"], f32)[:H_]),
        "w1_cn": np.ascontiguousarray(np.asarray(inputs["comb_w1"], f32)[H_:]),
        "w2_c": w2.astype(bf),
        "bvec": np.stack([
            np.asarray(inputs["self_b1"], f32),
            np.asarray(inputs["nb_b1"], f32),
            np.asarray(inputs["self_b2"], f32),
            np.asarray(inputs["nb_b2"], f32),
            np.asarray(inputs["comb_b1"], f32),
        ], axis=1),
        "id_f32": np.eye(128, dtype=f32),
        "w2bdpack": w2bdpack,
        "inveye": ive.astype(bf),
    }
    return consts


def _build_fast_path(nc):
    """Cache a single jitted shard_map executable so repeat kernel() calls
    skip jax re-tracing (same lowering run_bass_kernel_spmd uses under axon)."""
    import jax
    from jax.sharding import Mesh, PartitionSpec
    from jax.experimental.shard_map import shard_map

    bass2jax.install_neuronx_cc_hook()
    pname = nc.partition_id_tensor.name if nc.partition_id_tensor else None
    in_names, out_names, out_avals = [], [], []
    for alloc in nc.m.functions[0].allocations:
        if not isinstance(alloc, mybir.MemoryLocationSet):
            continue
        name = alloc.memorylocations[0].name
        if alloc.kind == "ExternalInput":
            if name != pname:
                in_names.append(name)
        elif alloc.kind == "ExternalOutput":
            out_names.append(name)
            out_avals.append(jax.core.ShapedArray(tuple(alloc.tensor_shape),
                                                  mybir.dt.np(alloc.dtype)))
    all_names = in_names + out_names + ([pname] if pname else [])

    def _body(*args):
        operands = list(args)
        if pname is not None:
            operands.append(bass2jax.partition_id_tensor())
        return tuple(bass2jax._bass_exec_p.bind(
            *operands, out_avals=tuple(out_avals), in_names=tuple(all_names),
            out_names=tuple(out_names), lowering_input_output_aliases=(),
            sim_require_finite=True, sim_require_nnan=True, nc=nc))

    devices = jax.devices()[:NCORES]
    mesh = Mesh(np.asarray(devices), ("core",))
    n_io = len(in_names) + len(out_names)
    sharded = jax.jit(
        shard_map(_body, mesh=mesh, in_specs=(PartitionSpec("core"),) * n_io,
                  out_specs=(PartitionSpec("core"),) * len(out_names),
                  check_rep=False),
        keep_unused=True,
    )
    return sharded, in_names, out_names, out_avals


def kernel(**inputs):
    first = "nc" not in _CACHE
    if first:
        _CACHE["nc"] = _build_module()
    nc = _CACHE["nc"]

    consts = _host_constants(inputs)
    nodes = np.asarray(inputs["nodes"], np.float32).reshape(B, N, D)
    edges = (np.asarray(inputs["edges"]) != 0).astype(np.uint8)

    in_maps = []
    for c in range(NCORES):
        m = dict(consts)
        m["nodes"] = np.ascontiguousarray(nodes[c])
        m["edges"] = edges[c]
        in_maps.append(m)

    if first:
        res = run_bass_kernel_spmd(nc, in_maps, core_ids=list(range(NCORES)))
        _CACHE["fast"] = _build_fast_path(nc)
        return np.stack([res.results[c]["out"] for c in range(NCORES)]).astype(np.float32)

    import jax
    sharded, in_names, out_names, out_avals = _CACHE["fast"]
    ckey = hash(tuple((k, v.tobytes()) for k, v in sorted(consts.items())))
    if _CACHE.get("ckey") != ckey:
        _CACHE["cdev"] = {
            n: jax.device_put(np.concatenate([np.asarray(in_maps[c][n])
                                              for c in range(NCORES)], axis=0))
            for n in in_names if n not in ("nodes", "edges")
        }
        _CACHE["zdev"] = [jax.device_put(np.zeros((NCORES * a.shape[0], *a.shape[1:]),
                                                  a.dtype)) for a in out_avals]
        _CACHE["ckey"] = ckey
    cdev = _CACHE["cdev"]
    concat_in = [cdev[n] if n in cdev else
                 np.concatenate([np.asarray(in_maps[c][n]) for c in range(NCORES)], axis=0)
                 for n in in_names]
    outs = sharded(*concat_in, *_CACHE["zdev"])
    i = out_names.index("out")
    return np.asarray(outs[i]).reshape(NCORES, N, H).astype(np.float32)


# revision 28
# speedup vs baseline: 1.1305x; 1.1305x over previous
"""GAT message-passing kernel for Trainium2 (8 NeuronCores, data-parallel over batch).

Math (per batch element b, derived from the reference nn.Module):
    x      = nodes.reshape(N, D)
    self_e = mlp2(x, self_*)                 # [N, H]
    nb_e   = mlp2(x, nb_*)                   # [N, H]
    U      = self_e @ comb_w1[:H]            # [N, H]  (i side)
    V      = nb_e @ comb_w1[H:] + comb_b1    # [N, H]  (j side)
    scores(i,j) = leaky(U_i + V_j) @ w2 + b2
                = 0.8*relu(U_i+V_j)@w2 + 0.2*(sU_i + sV_j) + const_i
    Softmax over j is invariant to per-i constants, so only
      s'(i,j) = 0.8*relu(U_i+V_j)@w2 + 0.2*sV_j  matters, and
      exp(s') factorizes as exp(0.8 relu(...)@w2) * exp(0.2 sV_j).
    ET[j,i]  = edges[j,i]*(j!=i) * exp(0.8 relu(U_i+V_j)@w2)
    den[i]   = sum_j ET[j,i]*esv_j      (esv_j = exp(0.2 sV_j))
    agg[i,:] = sum_j ET[j,i]*esv_j*nb_e[j,:]
    out[i]   = (den>eps) * (agg/den + self_e)
    (|scores| < 2, so exp needs no max-subtraction.)

Device mapping (one core per batch element):
  - Transposed (g,h)-on-partitions layout: partitions = (i-parity g, h), so one
    tensor_scalar(add,max) / activation(Relu,bias) op builds relu(V + U_i) for
    TWO i's at once as a [128, 512] tile.
  - PE reduces over (g,h) with slotted block-diagonal 0.8*w2 lhsT. Most pairs
    of slots go through ONE fp8 MatmulPerfMode.DoubleRow matmul (two K-planes,
    0.5 cycles/row -> 107 ns for 4 i's); DVE-built slots may instead use bf16
    single-slot matmuls (cheaper on DVE: 4x perf mode needs 2-byte dtypes).
    The per-pair engine/dtype assignment is tunable (GAT_PAIRS).
  - ACT applies exp straight out of PSUM (bf16); PE transposes 128x128 chunks;
    DVE/Pool multiply by mask tiles (edges * (1-eye), loaded via gpsimd
    cast-DMA u8->bf16) -> ET.
  - The per-j factor exp(0.2 sV_j) and the softmax denominator are folded into
    the aggregation matmul: rhs = [esv*nb_e | esv] (65 cols), so den arrives
    as PSUM column 64 already per-partition -- no row->column scatter.
  - Precompute MLP matmuls run as float32r (1 cycle/row at >=256 free dim,
    exact fp32 in sim); self_e / nb_e come from PE transposes of the already
    biased eT_s / eT_n (no extra matmuls).
  - fp8e4m3 quantization of the relu tiles + 0.8*w2 keeps absmax rel err
    ~9e-4 (measured offline vs fp64 reference).
"""

import os
import sys

sys.path.insert(0, "/opt/trn_rl_repo")

import numpy as np
import ml_dtypes

import concourse.bass as bass
import concourse.bacc as bacc
import concourse.tile as tile
from concourse import mybir, bass2jax
from concourse.bass_utils import run_bass_kernel_spmd

B, N, H, D = 8, 512, 64, 128
NCORES = 8
NT = N // 128          # 4 i/j tiles of 128
NPAIR = N // 2         # 256 i-pairs
F32 = mybir.dt.float32
F32R = mybir.dt.float32r
BF16 = mybir.dt.bfloat16
FP8 = mybir.dt.float8e4
I32 = mybir.dt.int32

# Per slot-pair engine assignment for the 128 pairs (4 it x 2 c x 16 t):
#   'b' = two bf16 builds on DVE + two bf16 single-slot matmuls
#   'v'/'a'/'p' = two fp8 builds on DVE/ACT/Pool + one fp8 DoubleRow matmul
# Either a 128-char string or comma counts like "b57,v12,a24,p35".
PAIR_SPEC = os.environ.get("GAT_PAIRS", "b55,v13,a25,p35")
# Engine for the 16 ET mask-multiplies (4 it x 4 jt): 'v' = DVE, 'p' = Pool.
ET_SPEC = os.environ.get("GAT_ETPAT", "v" * 16)

_CACHE = {}


def _expand_pairs(spec):
    if "," not in spec and len(spec) == 128:
        return spec
    counts = {}
    for part in spec.split(","):
        counts[part[0]] = int(part[1:])
    assert sum(counts.values()) == 128, counts
    # Bresenham-style proportional interleave for even engine spacing
    acc = {k: 0.0 for k in counts}
    out = []
    for _ in range(128):
        for k in acc:
            acc[k] += counts[k] / 128.0
        best = max(acc, key=lambda k: acc[k])
        acc[best] -= 1.0
        out.append(best)
    return "".join(out)


def _build_module():
    nc = bacc.Bacc("TRN2", target_bir_lowering=False, debug=False, num_devices=NCORES)

    # ---- per-core data ----
    nodes = nc.dram_tensor("nodes", [N, D], F32, kind="ExternalInput")
    edges = nc.dram_tensor("edges", [N, N], mybir.dt.uint8, kind="ExternalInput")
    # ---- packed host-prepared constants (same on all cores) ----
    # wpack128 [128, 258] = [w1_self(64) | w1_nb(64) | id_f32(128) | w2bd(2)]
    # wpack64  [64, 262]  = [w2_self | w2_nb | w1_cs | w1_cn | w2_c(1) | bvec(5)]
    wpack128 = nc.dram_tensor("wpack128", [128, 258], F32, kind="ExternalInput")
    wpack64 = nc.dram_tensor("wpack64", [H, 262], F32, kind="ExternalInput")
    inveye = nc.dram_tensor("inveye", [128, 128], BF16, kind="ExternalInput")

    out = nc.dram_tensor("out", [N, H], F32, kind="ExternalOutput")

    with tile.TileContext(nc) as tc:
        _emit(nc, tc, locals())
    nc.compile()
    return nc


def _emit(nc, tc, t):
    AF = mybir.ActivationFunctionType
    OP = mybir.AluOpType
    pairs = _expand_pairs(PAIR_SPEC)

    with (
        tc.tile_pool(name="persist", bufs=1) as P,
        tc.tile_pool(name="xwork", bufs=2) as XW,
        tc.tile_pool(name="ework", bufs=2) as EW,
        tc.tile_pool(name="relb", bufs=14) as RLB,
        tc.tile_pool(name="rel8", bufs=24) as RL8,
        tc.tile_pool(name="xexp", bufs=2) as XE,
        tc.tile_pool(name="xtr", bufs=4) as PXS,
        tc.tile_pool(name="etw", bufs=4) as ETW,
        tc.tile_pool(name="small", bufs=4) as SM,
        tc.tile_pool(name="psumR", bufs=2, space="PSUM") as PR,
        tc.tile_pool(name="psumT", bufs=3, space="PSUM") as PT,
        tc.tile_pool(name="psumM", bufs=1, space="PSUM") as PM,
        tc.tile_pool(name="psumC", bufs=1, space="PSUM") as PC,
        tc.tile_pool(name="psumA", bufs=1, space="PSUM") as PA,
    ):
        # ---------- load constants: one big DMA per engine queue ----------
        xin_all = XW.tile([128, NT, D], F32, tag="xin")
        nc.sync.dma_start(out=xin_all[:],
                          in_=t["nodes"].ap().rearrange("(t p) d -> p t d", p=128))
        wp128 = P.tile([128, 258], F32, tag="wp128")
        nc.scalar.dma_start(out=wp128[:], in_=t["wpack128"].ap())
        wp64 = P.tile([H, 262], F32, tag="wp64")
        nc.gpsimd.dma_start(out=wp64[:], in_=t["wpack64"].ap())
        ive = P.tile([128, 128], BF16, tag="ive")
        nc.gpsimd.dma_start(out=ive[:], in_=t["inveye"].ap())
        # masks: edges -> bf16 via one casting gpsimd DMA (issue early; the
        # transfer runs async on the DMA queues), diag zeroed later
        mask_all = P.tile([128, NT, N], BF16, tag="mask_all")
        nc.gpsimd.dma_start(out=mask_all[:],
                            in_=t["edges"].ap().rearrange("(t p) j -> p t j", p=128))
        masks = [mask_all[:, jt, :] for jt in range(NT)]

        wp128r = wp128.bitcast(F32R)
        wp64r = wp64.bitcast(F32R)
        w1s_r, w1n_r = wp128r[:, 0:64], wp128r[:, 64:128]
        idf = wp128[:, 128:256]
        idf64 = wp128[0:64, 128:192]
        idf1 = wp128[0:1, 128:129]
        w2bdf = wp128[:, 256:258]
        w2s_r, w2n_r = wp64r[:, 0:64], wp64r[:, 64:128]
        w1cs_r, w1cn_r = wp64r[:, 128:192], wp64r[:, 192:256]
        b1s, b1n = wp64[:, 257:258], wp64[:, 258:259]
        b2sc, b2nc, b1c = wp64[:, 259:260], wp64[:, 260:261], wp64[:, 261:262]
        w2cb = P.tile([H, 1], BF16, tag="w2cb")
        nc.vector.tensor_copy(out=w2cb[:], in_=wp64[:, 256:257])
        # bf16 block-diagonal buffer for single-slot matmuls
        w2bd_all = P.tile([128, 128], BF16, tag="w2bd_all")
        nc.gpsimd.memset(w2bd_all[:], 0.0)
        nc.vector.tensor_copy(out=w2bd_all[:, 62:64], in_=w2bdf)
        w2bd_sb = [w2bd_all[:, 62 - 2 * s:126 - 2 * s] for s in range(32)]
        # fp8 DoubleRow weights: plane 0 = window buf A (cols 62:64 hot),
        # plane 1 = window buf B (cols 64:66 hot); free layout [2, 130]
        w2dr = P.tile([128, 2, 130], FP8, tag="w2dr")
        nc.gpsimd.memset(w2dr[:], 0.0)
        nc.vector.tensor_copy(out=w2dr[:, 0, 62:64], in_=w2bdf)
        nc.vector.tensor_copy(out=w2dr[:, 1, 64:66], in_=w2bdf)

        # ---------- x -> x^T ----------
        xT = P.tile([D, N], F32, tag="xT")
        for it in range(NT):
            px = PT.tile([128, 128], F32, tag="pt", name="px", padded_shape=[128, 128])
            nc.tensor.transpose(px[:], xin_all[:, it, :], idf)
            nc.vector.tensor_copy(out=xT[:, bass.ts(it, 128)], in_=px[:])
        xTr = xT.bitcast(F32R)

        # ---------- tiny MLPs (transposed; h on partitions), f32r matmuls ----
        pm = PM.tile([128, N], F32, tag="mm", name="pm_n1")
        nc.tensor.matmul(pm[:H, :], w1n_r, xTr[:], start=True, stop=True)
        z = EW.tile([H, N], F32, tag="lk_z")
        nc.gpsimd.tensor_scalar_add(out=z[:], in0=pm[:H, :], scalar1=b1n)
        h1T_n = P.tile([H, N], F32, tag="h1T_n")
        nc.vector.scalar_tensor_tensor(out=h1T_n[:], in0=z[:], scalar=0.2,
                                       in1=z[:], op0=OP.mult, op1=OP.max)

        pm = PM.tile([128, N], F32, tag="mm", name="pm_n2")
        nc.tensor.matmul(pm[:H, :], w2n_r, h1T_n.bitcast(F32R)[:],
                         start=True, stop=True)
        eT_n = P.tile([H, N], F32, tag="eT_n")
        nc.gpsimd.tensor_scalar_add(out=eT_n[:], in0=pm[:H, :], scalar1=b2nc)

        # Vrep (bf16, both partition halves) straight from PSUM
        pm = PM.tile([128, N], F32, tag="mm", name="pm_n3")
        nc.tensor.matmul(pm[:H, :], w1cn_r, eT_n.bitcast(F32R)[:],
                         start=True, stop=True)
        Vrep = P.tile([128, N], BF16, tag="Vrep")
        nc.scalar.activation(out=Vrep[:H, :], in_=pm[:H, :], func=AF.Identity,
                             bias=b1c, scale=1.0)
        nc.vector.tensor_scalar_add(out=Vrep[H:, :], in0=pm[:H, :], scalar1=b1c)

        for jt in range(NT):
            nc.vector.tensor_mul(out=mask_all[:, jt, bass.ts(jt, 128)],
                                 in0=mask_all[:, jt, bass.ts(jt, 128)], in1=ive[:])

        # self chain, two 256-column chunks so U2's early columns land early
        h1T_s = P.tile([H, N], F32, tag="h1T_s")
        eT_s = P.tile([H, N], F32, tag="eT_s")
        U2 = P.tile([128, NPAIR], F32, tag="U2")
        for ch in range(2):
            cs = bass.ts(ch, 256)
            pc = PC.tile([128, 256], F32, tag="pc", name="pc1")
            nc.tensor.matmul(pc[:H, :], w1s_r, xTr[:, cs],
                             start=True, stop=True)
            zc = EW.tile([H, 256], F32, tag="lk_zc", name="zc")
            nc.gpsimd.tensor_scalar_add(out=zc[:], in0=pc[:H, :], scalar1=b1s)
            nc.vector.scalar_tensor_tensor(out=h1T_s[:, cs], in0=zc[:], scalar=0.2,
                                           in1=zc[:], op0=OP.mult, op1=OP.max)
            pc = PC.tile([128, 256], F32, tag="pc", name="pc2")
            nc.tensor.matmul(pc[:H, :], w2s_r, h1T_s.bitcast(F32R)[:, cs],
                             start=True, stop=True)
            nc.gpsimd.tensor_scalar_add(out=eT_s[:, cs], in0=pc[:H, :], scalar1=b2sc)
            pc = PC.tile([128, 256], F32, tag="pc", name="pc3")
            nc.tensor.matmul(pc[:H, :], w1cs_r, eT_s.bitcast(F32R)[:, cs],
                             start=True, stop=True)
            psplit = pc[:H, :].rearrange("p (i g) -> p i g", g=2)
            nc.gpsimd.tensor_scalar_add(out=U2[:H, bass.ts(ch, 128)],
                                        in0=psplit[:, :, 0], scalar1=0.0)
            nc.gpsimd.tensor_scalar_add(out=U2[H:, bass.ts(ch, 128)],
                                        in0=psplit[:, :, 1], scalar1=0.0)

        # exp(0.2 * sV) row -> [128, NT] per-partition scalars
        pm = PM.tile([128, N], F32, tag="mm", name="pm_sv")
        nc.tensor.matmul(pm[:1, :], w2cb[:], Vrep[:H, :], start=True, stop=True)
        sv_row = SM.tile([1, N], F32, tag="sv_row")
        nc.scalar.activation(out=sv_row[:], in_=pm[:1, :], func=AF.Exp, scale=0.2)
        pesv = PT.tile([128, 128], F32, tag="pt", name="pesv", padded_shape=[128, 128])
        for tq in range(NT):
            nc.tensor.transpose(pesv[:, tq:tq + 1], sv_row[:, bass.ts(tq, 128)],
                                idf1)
        esv = P.tile([128, NT], F32, tag="esv")
        nc.vector.tensor_copy(out=esv[:], in_=pesv[:, 0:NT])

        # ---------- self_e [i,H] via transpose of eT_s; nbe2 = [esv*nb_e|esv] --
        # (emitted inside the warmup window, after group(0, c0))
        selfe, nbe2 = [], []

        def emit_late_pre():
            for it in range(NT):
                pT = PT.tile([128, 128], F32, tag="pt", name="pTs",
                             padded_shape=[128, 128])
                nc.tensor.transpose(pT[:, 0:64], eT_s[:, bass.ts(it, 128)], idf64)
                se = P.tile([128, H], F32, tag=f"selfe{it}")
                nc.gpsimd.tensor_scalar_add(out=se[:], in0=pT[:, 0:64], scalar1=0.0)
                selfe.append(se)
            for jt in range(NT):
                pT = PT.tile([128, 128], F32, tag="pt", name="pTn",
                             padded_shape=[128, 128])
                nc.tensor.transpose(pT[:, 0:64], eT_n[:, bass.ts(jt, 128)], idf64)
                ne = P.tile([128, H + 1], BF16, tag=f"nbe{jt}")
                nc.gpsimd.tensor_scalar_mul(out=ne[:, 0:H], in0=pT[:, 0:64],
                                            scalar1=esv[:, jt:jt + 1])
                nc.gpsimd.tensor_copy(out=ne[:, H:H + 1], in_=esv[:, jt:jt + 1])
                nbe2.append(ne)

        # ---------- main pass: scores -> exp -> ET -> agg+den -> out ----------
        def emit_build(eng, out_ap, p):
            u = U2[:, p:p + 1]
            if eng == "v" or eng == "b":
                nc.vector.tensor_scalar(out=out_ap, in0=Vrep[:], scalar1=u,
                                        scalar2=0.0, op0=OP.add, op1=OP.max)
            elif eng == "a":
                nc.scalar.activation(out=out_ap, in_=Vrep[:], func=AF.Relu,
                                     bias=u, scale=1.0)
            else:
                nc.gpsimd.tensor_scalar(out=out_ap, in0=Vrep[:], scalar1=u,
                                        scalar2=0.0, op0=OP.add, op1=OP.max)

        def emit_group(it, c, ps):
            glist = [pairs[(it * 2 + c) * 16 + tt] for tt in range(16)]
            if it == NT - 1 and c == 1:
                # drain the slow build engines first so the tail is short
                order = sorted(range(16), key=lambda tt: "apvb".index(glist[tt]))
            else:
                order = list(range(16))
            first, last = order[0], order[-1]
            for tt in order:
                eng = glist[tt]
                p0 = 64 * it + 32 * c + 2 * tt
                if eng == "b":
                    for g in range(2):
                        rl = RLB.tile([128, N], BF16, tag="rlb")
                        emit_build("b", rl[:], p0 + g)
                        nc.tensor.matmul(ps[bass.ts(c, 64), :], w2bd_sb[2 * tt + g],
                                         rl[:], start=(tt == first and g == 0),
                                         stop=(tt == last and g == 1))
                else:
                    rl2 = RL8.tile([128, 2, N], FP8, tag="rl8")
                    emit_build(eng, rl2[:, 0, :], p0)
                    emit_build(eng, rl2[:, 1, :], p0 + 1)
                    nc.tensor.matmul(ps[bass.ts(c, 64), :],
                                     w2dr[:, :, 62 - 4 * tt:126 - 4 * tt], rl2[:],
                                     start=(tt == first), stop=(tt == last),
                                     perf_mode=mybir.MatmulPerfMode.DoubleRow)

        def emit_post(it, ps):
            X = XE.tile([128, N], BF16, tag="X")
            nc.scalar.activation(out=X[:], in_=ps[:], func=AF.Exp)
            pa = PA.tile([128, H + 1], F32, tag="pa", name="pa")
            for jt in range(NT):
                px = PXS.tile([128, 128], BF16, tag="pxs")
                nc.sync.dma_start_transpose(out=px[:], in_=X[:, bass.ts(jt, 128)])
                etw = ETW.tile([128, 128], BF16, tag="etw")
                eng_et = nc.gpsimd if ET_SPEC[it * NT + jt] == "p" else nc.vector
                eng_et.tensor_mul(out=etw[:], in0=px[:],
                                  in1=mask_all[:, jt, bass.ts(it, 128)])
                nc.tensor.matmul(pa[:], etw[:], nbe2[jt][:],
                                 start=(jt == 0), stop=(jt == NT - 1))
            den = pa[:, H:H + 1]
            # no-neighbor rows have agg == 0 exactly, so an ungated reciprocal
            # (1e30) still yields 0 for the agg term; only selfe needs the gate
            asm = nc.vector if it == NT - 1 else nc.gpsimd
            gate = SM.tile([128, 1], F32, tag="gate", name="gate")
            asm.tensor_single_scalar(out=gate[:], in_=den, scalar=1e-6, op=OP.is_gt)
            dsafe = SM.tile([128, 1], F32, tag="dsafe", name="dsafe")
            asm.tensor_scalar_max(out=dsafe[:], in0=den, scalar1=1e-30)
            recip = SM.tile([128, 1], F32, tag="recip", name="recip")
            nc.vector.reciprocal(out=recip[:], in_=dsafe[:])
            sg = SM.tile([128, H], F32, tag="sg")
            nc.vector.tensor_scalar_mul(out=sg[:], in0=selfe[it][:], scalar1=gate[:])
            ot = SM.tile([128, H], F32, tag="ot")
            nc.vector.scalar_tensor_tensor(out=ot[:], in0=pa[:, 0:H],
                                           scalar=recip[:], in1=sg[:],
                                           op0=OP.mult, op1=OP.add)
            nc.sync.dma_start(out=t["out"].ap()[bass.ts(it, 128), :], in_=ot[:])

        ps_tiles = [None] * NT
        for it in range(NT):
            ps = PR.tile([128, N], F32, tag="psumR", name=f"ps{it}")
            ps_tiles[it] = ps
            emit_group(it, 0, ps)
            if it == 0:
                emit_late_pre()
            else:
                emit_post(it - 1, ps_tiles[it - 1])
            emit_group(it, 1, ps)
        emit_post(NT - 1, ps_tiles[NT - 1])


def _host_constants(inputs):
    f32 = np.float32
    bf = ml_dtypes.bfloat16
    H_ = H
    w2 = np.asarray(inputs["comb_w2"], f32)            # [H, 1]
    w2bdpack = np.zeros((128, 2), f32)
    w2bdpack[0:H_, 0] = 0.8 * w2[:, 0]
    w2bdpack[H_:128, 1] = 0.8 * w2[:, 0]
    ive = (1.0 - np.eye(128)).astype(f32)
    wpack128 = np.concatenate([
        np.asarray(inputs["self_w1"], f32),          # [128, 64]
        np.asarray(inputs["nb_w1"], f32),            # [128, 64]
        np.eye(128, dtype=f32),                      # [128, 128]
        w2bdpack,                                    # [128, 2]
    ], axis=1)
    bvec = np.stack([
        np.asarray(inputs["self_b1"], f32),
        np.asarray(inputs["nb_b1"], f32),
        np.asarray(inputs["self_b2"], f32),
        np.asarray(inputs["nb_b2"], f32),
        np.asarray(inputs["comb_b1"], f32),
    ], axis=1)                                       # [64, 5]
    wpack64 = np.concatenate([
        np.asarray(inputs["self_w2"], f32),
        np.asarray(inputs["nb_w2"], f32),
        np.ascontiguousarray(np.asarray(inputs["comb_w1"], f32)[:H_]),
        np.ascontiguousarray(np.asarray(inputs["comb_w1"], f32)[H_:]),
        w2,                                          # [64, 1]
        bvec,
    ], axis=1)
    consts = {
        "wpack128": wpack128,
        "wpack64": wpack64,
        "inveye": ive.astype(bf),
    }
    return consts


def _build_fast_path(nc):
    """Cache a single jitted shard_map executable so repeat kernel() calls
    skip jax re-tracing (same lowering run_bass_kernel_spmd uses under axon)."""
    import jax
    from jax.sharding import Mesh, PartitionSpec
    from jax.experimental.shard_map import shard_map

    bass2jax.install_neuronx_cc_hook()
    pname = nc.partition_id_tensor.name if nc.partition_id_tensor else None
    in_names, out_names, out_avals = [], [], []
    for alloc in nc.m.functions[0].allocations:
        if not isinstance(alloc, mybir.MemoryLocationSet):
            continue
        name = alloc.memorylocations[0].name
        if alloc.kind == "ExternalInput":
            if name != pname:
                in_names.append(name)
        elif alloc.kind == "ExternalOutput":
            out_names.append(name)
            out_avals.append(jax.core.ShapedArray(tuple(alloc.tensor_shape),
                                                  mybir.dt.np(alloc.dtype)))
    all_names = in_names + out_names + ([pname] if pname else [])

    def _body(*args):
        operands = list(args)
        if pname is not None:
            operands.append(bass2jax.partition_id_tensor())
        return tuple(bass2jax._bass_exec_p.bind(
            *operands, out_avals=tuple(out_avals), in_names=tuple(all_names),
            out_names=tuple(out_names), lowering_input_output_aliases=(),
            sim_require_finite=True, sim_require_nnan=True, nc=nc))

    devices = jax.devices()[:NCORES]
    mesh = Mesh(np.asarray(devices), ("core",))
    n_io = len(in_names) + len(out_names)
    sharded = jax.jit(
        shard_map(_body, mesh=mesh, in_specs=(PartitionSpec("core"),) * n_io,
                  out_specs=(PartitionSpec("core"),) * len(out_names),
                  check_rep=False),
        keep_unused=True,
    )
    return sharded, in_names, out_names, out_avals


def kernel(**inputs):
    first = "nc" not in _CACHE
    if first:
        _CACHE["nc"] = _build_module()
    nc = _CACHE["nc"]

    consts = _host_constants(inputs)
    nodes = np.asarray(inputs["nodes"], np.float32).reshape(B, N, D)
    edges = (np.asarray(inputs["edges"]) != 0).astype(np.uint8)

    in_maps = []
    for c in range(NCORES):
        m = dict(consts)
        m["nodes"] = np.ascontiguousarray(nodes[c])
        m["edges"] = edges[c]
        in_maps.append(m)

    if first:
        res = run_bass_kernel_spmd(nc, in_maps, core_ids=list(range(NCORES)))
        _CACHE["fast"] = _build_fast_path(nc)
        return np.stack([res.results[c]["out"] for c in range(NCORES)]).astype(np.float32)

    import jax
    sharded, in_names, out_names, out_avals = _CACHE["fast"]
    ckey = hash(tuple((k, v.tobytes()) for k, v in sorted(consts.items())))
    if _CACHE.get("ckey") != ckey:
        _CACHE["cdev"] = {
            n: jax.device_put(np.concatenate([np.asarray(in_maps[c][n])
                                              for c in range(NCORES)], axis=0))
            for n in in_names if n not in ("nodes", "edges")
        }
        _CACHE["zdev"] = [jax.device_put(np.zeros((NCORES * a.shape[0], *a.shape[1:]),
                                                  a.dtype)) for a in out_avals]
        _CACHE["ckey"] = ckey
    cdev = _CACHE["cdev"]
    concat_in = [cdev[n] if n in cdev else
                 np.concatenate([np.asarray(in_maps[c][n]) for c in range(NCORES)], axis=0)
                 for n in in_names]
    outs = sharded(*concat_in, *_CACHE["zdev"])
    i = out_names.index("out")
    return np.asarray(outs[i]).reshape(NCORES, N, H).astype(np.float32)


# revision 29
# speedup vs baseline: 1.1818x; 1.0453x over previous
"""GAT message-passing kernel for Trainium2 (8 NeuronCores, data-parallel over batch).

Math (per batch element b, derived from the reference nn.Module):
    x      = nodes.reshape(N, D)
    self_e = mlp2(x, self_*)                 # [N, H]
    nb_e   = mlp2(x, nb_*)                   # [N, H]
    U      = self_e @ comb_w1[:H]            # [N, H]  (i side)
    V      = nb_e @ comb_w1[H:] + comb_b1    # [N, H]  (j side)
    scores(i,j) = leaky(U_i + V_j) @ w2 + b2
                = 0.8*relu(U_i+V_j)@w2 + 0.2*(sU_i + sV_j) + const_i
    Softmax over j is invariant to per-i constants, so only
      s'(i,j) = 0.8*relu(U_i+V_j)@w2 + 0.2*sV_j  matters, and
      exp(s') factorizes as exp(0.8 relu(...)@w2) * exp(0.2 sV_j).
    ET[j,i]  = edges[j,i]*(j!=i) * exp(0.8 relu(U_i+V_j)@w2)
    den[i]   = sum_j ET[j,i]*esv_j      (esv_j = exp(0.2 sV_j))
    agg[i,:] = sum_j ET[j,i]*esv_j*nb_e[j,:]
    out[i]   = (den>eps) * (agg/den + self_e)
    (|scores| < 2, so exp needs no max-subtraction.)

Device mapping (one core per batch element):
  - Transposed (g,h)-on-partitions layout: partitions = (i-parity g, h), so one
    tensor_scalar(add,max) / activation(Relu,bias) op builds relu(V + U_i) for
    TWO i's at once as a [128, 512] tile.
  - PE reduces over (g,h) with slotted block-diagonal 0.8*w2 lhsT. Most pairs
    of slots go through ONE fp8 MatmulPerfMode.DoubleRow matmul (two K-planes,
    0.5 cycles/row -> 107 ns for 4 i's); DVE-built slots may instead use bf16
    single-slot matmuls (cheaper on DVE: 4x perf mode needs 2-byte dtypes).
    The per-pair engine/dtype assignment is tunable (GAT_PAIRS).
  - ACT applies exp straight out of PSUM (bf16); PE transposes 128x128 chunks;
    DVE/Pool multiply by mask tiles (edges * (1-eye), loaded via gpsimd
    cast-DMA u8->bf16) -> ET.
  - The per-j factor exp(0.2 sV_j) and the softmax denominator are folded into
    the aggregation matmul: rhs = [esv*nb_e | esv] (65 cols), so den arrives
    as PSUM column 64 already per-partition -- no row->column scatter.
  - Precompute MLP matmuls run as float32r (1 cycle/row at >=256 free dim,
    exact fp32 in sim); self_e / nb_e come from PE transposes of the already
    biased eT_s / eT_n (no extra matmuls).
  - fp8e4m3 quantization of the relu tiles + 0.8*w2 keeps absmax rel err
    ~9e-4 (measured offline vs fp64 reference).
"""

import os
import sys

sys.path.insert(0, "/opt/trn_rl_repo")

import numpy as np
import ml_dtypes

import concourse.bass as bass
import concourse.bacc as bacc
import concourse.tile as tile
from concourse import mybir, bass2jax
from concourse.bass_utils import run_bass_kernel_spmd

B, N, H, D = 8, 512, 64, 128
NCORES = 8
NT = N // 128          # 4 i/j tiles of 128
NPAIR = N // 2         # 256 i-pairs
F32 = mybir.dt.float32
F32R = mybir.dt.float32r
BF16 = mybir.dt.bfloat16
FP8 = mybir.dt.float8e4
I32 = mybir.dt.int32

# Per slot-pair engine assignment for the 128 pairs (4 it x 2 c x 16 t):
#   'b' = two bf16 builds on DVE + two bf16 single-slot matmuls
#   'v'/'a'/'p' = two fp8 builds on DVE/ACT/Pool + one fp8 DoubleRow matmul
# Either a 128-char string or comma counts like "b57,v12,a24,p35".
PAIR_SPEC = os.environ.get("GAT_PAIRS", "b55,v13,a25,p35")
# Engine for the 16 ET mask-multiplies (4 it x 4 jt): 'v' = DVE, 'p' = Pool.
ET_SPEC = os.environ.get("GAT_ETPAT", "v" * 16)

_CACHE = {}


def _expand_pairs(spec):
    if "," not in spec and len(spec) == 128:
        return spec
    counts = {}
    for part in spec.split(","):
        counts[part[0]] = int(part[1:])
    assert sum(counts.values()) == 128, counts
    # Bresenham-style proportional interleave for even engine spacing
    acc = {k: 0.0 for k in counts}
    out = []
    for _ in range(128):
        for k in acc:
            acc[k] += counts[k] / 128.0
        best = max(acc, key=lambda k: acc[k])
        acc[best] -= 1.0
        out.append(best)
    return "".join(out)


def _build_module():
    nc = bacc.Bacc("TRN2", target_bir_lowering=False, debug=False, num_devices=NCORES)

    # ---- per-core data ----
    nodes = nc.dram_tensor("nodes", [N, D], F32, kind="ExternalInput")
    edges = nc.dram_tensor("edges", [N, N], mybir.dt.uint8, kind="ExternalInput")
    # ---- packed host-prepared constants (same on all cores) ----
    # wpack128 [128, 258] = [w1_self(64) | w1_nb(64) | id_f32(128) | w2bd(2)]
    # wpack64  [64, 262]  = [w2_self | w2_nb | w1_cs | w1_cn | w2_c(1) | bvec(5)]
    wpack128 = nc.dram_tensor("wpack128", [128, 258], F32, kind="ExternalInput")
    wpack64 = nc.dram_tensor("wpack64", [H, 262], F32, kind="ExternalInput")
    inveye = nc.dram_tensor("inveye", [128, 128], BF16, kind="ExternalInput")

    out = nc.dram_tensor("out", [N, H], F32, kind="ExternalOutput")

    with tile.TileContext(nc) as tc:
        _emit(nc, tc, locals())
    nc.compile()
    return nc


def _emit(nc, tc, t):
    AF = mybir.ActivationFunctionType
    OP = mybir.AluOpType
    pairs = _expand_pairs(PAIR_SPEC)

    with (
        tc.tile_pool(name="persist", bufs=1) as P,
        tc.tile_pool(name="xwork", bufs=2) as XW,
        tc.tile_pool(name="ework", bufs=2) as EW,
        tc.tile_pool(name="relb", bufs=14) as RLB,
        tc.tile_pool(name="rel8", bufs=24) as RL8,
        tc.tile_pool(name="xexp", bufs=2) as XE,
        tc.tile_pool(name="xtr", bufs=4) as PXS,
        tc.tile_pool(name="etw", bufs=4) as ETW,
        tc.tile_pool(name="small", bufs=4) as SM,
        tc.tile_pool(name="psumR", bufs=2, space="PSUM") as PR,
        tc.tile_pool(name="psumT", bufs=3, space="PSUM") as PT,
        tc.tile_pool(name="psumM", bufs=1, space="PSUM") as PM,
        tc.tile_pool(name="psumC", bufs=1, space="PSUM") as PC,
        tc.tile_pool(name="psumA", bufs=1, space="PSUM") as PA,
    ):
        # ---------- load constants: one big DMA per engine queue ----------
        xin_all = XW.tile([128, NT, D], F32, tag="xin")
        nc.sync.dma_start(out=xin_all[:],
                          in_=t["nodes"].ap().rearrange("(t p) d -> p t d", p=128))
        wp128 = P.tile([128, 258], F32, tag="wp128")
        nc.scalar.dma_start(out=wp128[:], in_=t["wpack128"].ap())
        wp64 = P.tile([H, 262], F32, tag="wp64")
        nc.gpsimd.dma_start(out=wp64[:], in_=t["wpack64"].ap())
        ive = P.tile([128, 128], BF16, tag="ive")
        nc.gpsimd.dma_start(out=ive[:], in_=t["inveye"].ap())
        # masks: edges -> bf16 via one casting gpsimd DMA (issue early; the
        # transfer runs async on the DMA queues), diag zeroed later
        mask_all = P.tile([128, NT, N], BF16, tag="mask_all")
        nc.gpsimd.dma_start(out=mask_all[:],
                            in_=t["edges"].ap().rearrange("(t p) j -> p t j", p=128))
        masks = [mask_all[:, jt, :] for jt in range(NT)]

        wp128r = wp128.bitcast(F32R)
        wp64r = wp64.bitcast(F32R)
        w1s_r, w1n_r = wp128r[:, 0:64], wp128r[:, 64:128]
        idf = wp128[:, 128:256]
        idf64 = wp128[0:64, 128:192]
        idf1 = wp128[0:1, 128:129]
        w2bdf = wp128[:, 256:258]
        w2s_r, w2n_r = wp64r[:, 0:64], wp64r[:, 64:128]
        w1cs_r, w1cn_r = wp64r[:, 128:192], wp64r[:, 192:256]
        b1s, b1n = wp64[:, 257:258], wp64[:, 258:259]
        b2sc, b2nc, b1c = wp64[:, 259:260], wp64[:, 260:261], wp64[:, 261:262]
        w2cb = P.tile([H, 1], BF16, tag="w2cb")
        nc.vector.tensor_copy(out=w2cb[:], in_=wp64[:, 256:257])
        # bf16 block-diagonal buffer for single-slot matmuls
        w2bd_all = P.tile([128, 128], BF16, tag="w2bd_all")
        nc.gpsimd.memset(w2bd_all[:], 0.0)
        nc.vector.tensor_copy(out=w2bd_all[:, 62:64], in_=w2bdf)
        w2bd_sb = [w2bd_all[:, 62 - 2 * s:126 - 2 * s] for s in range(32)]
        # fp8 DoubleRow weights: plane 0 = window buf A (cols 62:64 hot),
        # plane 1 = window buf B (cols 64:66 hot); free layout [2, 130]
        w2dr = P.tile([128, 2, 130], FP8, tag="w2dr")
        nc.gpsimd.memset(w2dr[:], 0.0)
        nc.vector.tensor_copy(out=w2dr[:, 0, 62:64], in_=w2bdf)
        nc.vector.tensor_copy(out=w2dr[:, 1, 64:66], in_=w2bdf)

        # ---------- x -> x^T ----------
        xT = P.tile([D, N], F32, tag="xT")
        for it in range(NT):
            px = PT.tile([128, 128], F32, tag="pt", name="px", padded_shape=[128, 128])
            nc.tensor.transpose(px[:], xin_all[:, it, :], idf)
            nc.vector.tensor_copy(out=xT[:, bass.ts(it, 128)], in_=px[:])
        xTr = xT.bitcast(F32R)

        # ---------- tiny MLPs (transposed; h on partitions), f32r matmuls ----
        pm = PM.tile([128, N], F32, tag="mm", name="pm_n1")
        nc.tensor.matmul(pm[:H, :], w1n_r, xTr[:], start=True, stop=True)
        z = EW.tile([H, N], F32, tag="lk_z")
        nc.gpsimd.tensor_scalar_add(out=z[:], in0=pm[:H, :], scalar1=b1n)
        h1T_n = P.tile([H, N], F32, tag="h1T_n")
        nc.vector.scalar_tensor_tensor(out=h1T_n[:], in0=z[:], scalar=0.2,
                                       in1=z[:], op0=OP.mult, op1=OP.max)

        pm = PM.tile([128, N], F32, tag="mm", name="pm_n2")
        nc.tensor.matmul(pm[:H, :], w2n_r, h1T_n.bitcast(F32R)[:],
                         start=True, stop=True)
        eT_n = P.tile([H, N], F32, tag="eT_n")
        nc.gpsimd.tensor_scalar_add(out=eT_n[:], in0=pm[:H, :], scalar1=b2nc)

        # Vrep (bf16, both partition halves) straight from PSUM
        pm = PM.tile([128, N], F32, tag="mm", name="pm_n3")
        nc.tensor.matmul(pm[:H, :], w1cn_r, eT_n.bitcast(F32R)[:],
                         start=True, stop=True)
        Vrep = P.tile([128, N], BF16, tag="Vrep")
        nc.scalar.activation(out=Vrep[:H, :], in_=pm[:H, :], func=AF.Identity,
                             bias=b1c, scale=1.0)
        nc.vector.tensor_scalar_add(out=Vrep[H:, :], in0=pm[:H, :], scalar1=b1c)

        for jt in range(NT):
            nc.vector.tensor_mul(out=mask_all[:, jt, bass.ts(jt, 128)],
                                 in0=mask_all[:, jt, bass.ts(jt, 128)], in1=ive[:])

        # self chain, two 256-column chunks so U2's early columns land early
        h1T_s = P.tile([H, N], F32, tag="h1T_s")
        eT_s = P.tile([H, N], F32, tag="eT_s")
        U2 = P.tile([128, NPAIR], F32, tag="U2")
        for ch in range(2):
            cs = bass.ts(ch, 256)
            pc = PC.tile([128, 256], F32, tag="pc", name="pc1")
            nc.tensor.matmul(pc[:H, :], w1s_r, xTr[:, cs],
                             start=True, stop=True)
            zc = EW.tile([H, 256], F32, tag="lk_zc", name="zc")
            nc.gpsimd.tensor_scalar_add(out=zc[:], in0=pc[:H, :], scalar1=b1s)
            nc.vector.scalar_tensor_tensor(out=h1T_s[:, cs], in0=zc[:], scalar=0.2,
                                           in1=zc[:], op0=OP.mult, op1=OP.max)
            pc = PC.tile([128, 256], F32, tag="pc", name="pc2")
            nc.tensor.matmul(pc[:H, :], w2s_r, h1T_s.bitcast(F32R)[:, cs],
                             start=True, stop=True)
            nc.gpsimd.tensor_scalar_add(out=eT_s[:, cs], in0=pc[:H, :], scalar1=b2sc)
            pc = PC.tile([128, 256], F32, tag="pc", name="pc3")
            nc.tensor.matmul(pc[:H, :], w1cs_r, eT_s.bitcast(F32R)[:, cs],
                             start=True, stop=True)
            psplit = pc[:H, :].rearrange("p (i g) -> p i g", g=2)
            nc.gpsimd.tensor_scalar_add(out=U2[:H, bass.ts(ch, 128)],
                                        in0=psplit[:, :, 0], scalar1=0.0)
            nc.gpsimd.tensor_scalar_add(out=U2[H:, bass.ts(ch, 128)],
                                        in0=psplit[:, :, 1], scalar1=0.0)

        # exp(0.2 * sV) row -> [128, NT] per-partition scalars
        pm = PM.tile([128, N], F32, tag="mm", name="pm_sv")
        nc.tensor.matmul(pm[:1, :], w2cb[:], Vrep[:H, :], start=True, stop=True)
        sv_row = SM.tile([1, N], F32, tag="sv_row")
        nc.scalar.activation(out=sv_row[:], in_=pm[:1, :], func=AF.Exp, scale=0.2)
        pesv = PT.tile([128, 128], F32, tag="pt", name="pesv", padded_shape=[128, 128])
        for tq in range(NT):
            nc.tensor.transpose(pesv[:, tq:tq + 1], sv_row[:, bass.ts(tq, 128)],
                                idf1)
        esv = P.tile([128, NT], F32, tag="esv")
        nc.vector.tensor_copy(out=esv[:], in_=pesv[:, 0:NT])

        # ---------- self_e [i,H] via transpose of eT_s; nbe2 = [esv*nb_e|esv] --
        # (emitted inside the warmup window, after group(0, c0))
        selfe, nbe2 = [], []

        def emit_late_pre():
            for it in range(NT):
                pT = PT.tile([128, 128], F32, tag="pt", name="pTs",
                             padded_shape=[128, 128])
                nc.tensor.transpose(pT[:, 0:64], eT_s[:, bass.ts(it, 128)], idf64)
                se = P.tile([128, H], F32, tag=f"selfe{it}")
                nc.gpsimd.tensor_scalar_add(out=se[:], in0=pT[:, 0:64], scalar1=0.0)
                selfe.append(se)
            for jt in range(NT):
                pT = PT.tile([128, 128], F32, tag="pt", name="pTn",
                             padded_shape=[128, 128])
                nc.tensor.transpose(pT[:, 0:64], eT_n[:, bass.ts(jt, 128)], idf64)
                ne = P.tile([128, H + 1], BF16, tag=f"nbe{jt}")
                nc.gpsimd.tensor_scalar_mul(out=ne[:, 0:H], in0=pT[:, 0:64],
                                            scalar1=esv[:, jt:jt + 1])
                nc.gpsimd.tensor_copy(out=ne[:, H:H + 1], in_=esv[:, jt:jt + 1])
                nbe2.append(ne)

        # ---------- main pass: scores -> exp -> ET -> agg+den -> out ----------
        def emit_build(eng, out_ap, p):
            u = U2[:, p:p + 1]
            if eng == "v" or eng == "b":
                nc.vector.tensor_scalar(out=out_ap, in0=Vrep[:], scalar1=u,
                                        scalar2=0.0, op0=OP.add, op1=OP.max)
            elif eng == "a":
                nc.scalar.activation(out=out_ap, in_=Vrep[:], func=AF.Relu,
                                     bias=u, scale=1.0)
            else:
                nc.gpsimd.tensor_scalar(out=out_ap, in0=Vrep[:], scalar1=u,
                                        scalar2=0.0, op0=OP.add, op1=OP.max)

        cum = {"b": 0, "v": 0, "a": 0, "p": 0}
        COST = {"b": 388, "v": 654, "a": 1224, "p": 854}

        def emit_group(it, c, ps):
            glist = [pairs[(it * 2 + c) * 16 + tt] for tt in range(16)]
            # emit pairs in predicted build-arrival order so the in-order PE
            # queue never blocks an early build behind a late one
            arr = []
            for tt in range(16):
                cum[glist[tt]] += 1
                arr.append((cum[glist[tt]] * COST[glist[tt]], tt))
            order = [tt for _, tt in sorted(arr)]
            first, last = order[0], order[-1]
            for tt in order:
                eng = glist[tt]
                p0 = 64 * it + 32 * c + 2 * tt
                if eng == "b":
                    for g in range(2):
                        rl = RLB.tile([128, N], BF16, tag="rlb")
                        emit_build("b", rl[:], p0 + g)
                        nc.tensor.matmul(ps[bass.ts(c, 64), :], w2bd_sb[2 * tt + g],
                                         rl[:], start=(tt == first and g == 0),
                                         stop=(tt == last and g == 1))
                else:
                    rl2 = RL8.tile([128, 2, N], FP8, tag="rl8")
                    emit_build(eng, rl2[:, 0, :], p0)
                    emit_build(eng, rl2[:, 1, :], p0 + 1)
                    nc.tensor.matmul(ps[bass.ts(c, 64), :],
                                     w2dr[:, :, 62 - 4 * tt:126 - 4 * tt], rl2[:],
                                     start=(tt == first), stop=(tt == last),
                                     perf_mode=mybir.MatmulPerfMode.DoubleRow)

        def emit_post(it, ps):
            X = XE.tile([128, N], BF16, tag="X")
            nc.scalar.activation(out=X[:], in_=ps[:], func=AF.Exp)
            pa = PA.tile([128, H + 1], F32, tag="pa", name="pa")
            for jt in range(NT):
                px = PXS.tile([128, 128], BF16, tag="pxs")
                nc.sync.dma_start_transpose(out=px[:], in_=X[:, bass.ts(jt, 128)])
                etw = ETW.tile([128, 128], BF16, tag="etw")
                eng_et = nc.gpsimd if ET_SPEC[it * NT + jt] == "p" else nc.vector
                eng_et.tensor_mul(out=etw[:], in0=px[:],
                                  in1=mask_all[:, jt, bass.ts(it, 128)])
                nc.tensor.matmul(pa[:], etw[:], nbe2[jt][:],
                                 start=(jt == 0), stop=(jt == NT - 1))
            den = pa[:, H:H + 1]
            # no-neighbor rows have agg == 0 exactly, so an ungated reciprocal
            # (1e30) still yields 0 for the agg term; only selfe needs the gate
            asm = nc.vector if it == NT - 1 else nc.gpsimd
            gate = SM.tile([128, 1], F32, tag="gate", name="gate")
            asm.tensor_single_scalar(out=gate[:], in_=den, scalar=1e-6, op=OP.is_gt)
            dsafe = SM.tile([128, 1], F32, tag="dsafe", name="dsafe")
            asm.tensor_scalar_max(out=dsafe[:], in0=den, scalar1=1e-30)
            recip = SM.tile([128, 1], F32, tag="recip", name="recip")
            nc.vector.reciprocal(out=recip[:], in_=dsafe[:])
            sg = SM.tile([128, H], F32, tag="sg")
            nc.vector.tensor_scalar_mul(out=sg[:], in0=selfe[it][:], scalar1=gate[:])
            ot = SM.tile([128, H], F32, tag="ot")
            nc.vector.scalar_tensor_tensor(out=ot[:], in0=pa[:, 0:H],
                                           scalar=recip[:], in1=sg[:],
                                           op0=OP.mult, op1=OP.add)
            nc.sync.dma_start(out=t["out"].ap()[bass.ts(it, 128), :], in_=ot[:])

        def emit_post_half(it, ps, c):
            # 64-row half post-chain: lets the c=0 half run under the c=1 builds
            cs = bass.ts(c, 64)
            Xh = XE.tile([64, N], BF16, tag=f"Xh{c}", name=f"Xh{c}")
            nc.scalar.activation(out=Xh[:], in_=ps[cs, :], func=AF.Exp)
            pa = pa_last
            for jt in range(NT):
                px = PXS.tile([128, 64], BF16, tag="pxh")
                nc.sync.dma_start_transpose(out=px[:], in_=Xh[:, bass.ts(jt, 128)])
                etw = ETW.tile([128, 64], BF16, tag="etwh")
                nc.vector.tensor_mul(
                    out=etw[:], in0=px[:],
                    in1=mask_all[:, jt, 128 * it + 64 * c:128 * it + 64 * c + 64])
                nc.tensor.matmul(pa[cs, :], etw[:], nbe2[jt][:],
                                 start=(jt == 0), stop=(jt == NT - 1))
            den = pa[cs, H:H + 1]
            gate = SM.tile([64, 1], F32, tag="gateh", name="gateh")
            nc.vector.tensor_single_scalar(out=gate[:], in_=den, scalar=1e-6,
                                           op=OP.is_gt)
            dsafe = SM.tile([64, 1], F32, tag="dsafeh", name="dsafeh")
            nc.vector.tensor_scalar_max(out=dsafe[:], in0=den, scalar1=1e-30)
            recip = SM.tile([64, 1], F32, tag="reciph", name="reciph")
            nc.vector.reciprocal(out=recip[:], in_=dsafe[:])
            sg = SM.tile([64, H], F32, tag="sgh")
            nc.vector.tensor_scalar_mul(out=sg[:], in0=selfe[it][cs, :], scalar1=gate[:])
            ot = SM.tile([64, H], F32, tag="oth")
            nc.vector.scalar_tensor_tensor(out=ot[:], in0=pa[cs, 0:H],
                                           scalar=recip[:], in1=sg[:],
                                           op0=OP.mult, op1=OP.add)
            nc.sync.dma_start(out=t["out"].ap()[128 * it + 64 * c:128 * it + 64 * c + 64, :],
                              in_=ot[:])

        ps_tiles = [None] * NT
        pa_last = None
        for it in range(NT):
            ps = PR.tile([128, N], F32, tag="psumR", name=f"ps{it}")
            ps_tiles[it] = ps
            emit_group(it, 0, ps)
            if it == 0:
                emit_late_pre()
            else:
                emit_post(it - 1, ps_tiles[it - 1])
            if it == NT - 1:
                pa_last = PA.tile([128, H + 1], F32, tag="pa", name="pa_last")
                emit_post_half(it, ps, 0)
            emit_group(it, 1, ps)
        emit_post_half(NT - 1, ps_tiles[NT - 1], 1)


def _host_constants(inputs):
    f32 = np.float32
    bf = ml_dtypes.bfloat16
    H_ = H
    w2 = np.asarray(inputs["comb_w2"], f32)            # [H, 1]
    w2bdpack = np.zeros((128, 2), f32)
    w2bdpack[0:H_, 0] = 0.8 * w2[:, 0]
    w2bdpack[H_:128, 1] = 0.8 * w2[:, 0]
    ive = (1.0 - np.eye(128)).astype(f32)
    wpack128 = np.concatenate([
        np.asarray(inputs["self_w1"], f32),          # [128, 64]
        np.asarray(inputs["nb_w1"], f32),            # [128, 64]
        np.eye(128, dtype=f32),                      # [128, 128]
        w2bdpack,                                    # [128, 2]
    ], axis=1)
    bvec = np.stack([
        np.asarray(inputs["self_b1"], f32),
        np.asarray(inputs["nb_b1"], f32),
        np.asarray(inputs["self_b2"], f32),
        np.asarray(inputs["nb_b2"], f32),
        np.asarray(inputs["comb_b1"], f32),
    ], axis=1)                                       # [64, 5]
    wpack64 = np.concatenate([
        np.asarray(inputs["self_w2"], f32),
        np.asarray(inputs["nb_w2"], f32),
        np.ascontiguousarray(np.asarray(inputs["comb_w1"], f32)[:H_]),
        np.ascontiguousarray(np.asarray(inputs["comb_w1"], f32)[H_:]),
        w2,                                          # [64, 1]
        bvec,
    ], axis=1)
    consts = {
        "wpack128": wpack128,
        "wpack64": wpack64,
        "inveye": ive.astype(bf),
    }
    return consts


def _build_fast_path(nc):
    """Cache a single jitted shard_map executable so repeat kernel() calls
    skip jax re-tracing (same lowering run_bass_kernel_spmd uses under axon)."""
    import jax
    from jax.sharding import Mesh, PartitionSpec
    from jax.experimental.shard_map import shard_map

    bass2jax.install_neuronx_cc_hook()
    pname = nc.partition_id_tensor.name if nc.partition_id_tensor else None
    in_names, out_names, out_avals = [], [], []
    for alloc in nc.m.functions[0].allocations:
        if not isinstance(alloc, mybir.MemoryLocationSet):
            continue
        name = alloc.memorylocations[0].name
        if alloc.kind == "ExternalInput":
            if name != pname:
                in_names.append(name)
        elif alloc.kind == "ExternalOutput":
            out_names.append(name)
            out_avals.append(jax.core.ShapedArray(tuple(alloc.tensor_shape),
                                                  mybir.dt.np(alloc.dtype)))
    all_names = in_names + out_names + ([pname] if pname else [])

    def _body(*args):
        operands = list(args)
        if pname is not None:
            operands.append(bass2jax.partition_id_tensor())
        return tuple(bass2jax._bass_exec_p.bind(
            *operands, out_avals=tuple(out_avals), in_names=tuple(all_names),
            out_names=tuple(out_names), lowering_input_output_aliases=(),
            sim_require_finite=True, sim_require_nnan=True, nc=nc))

    devices = jax.devices()[:NCORES]
    mesh = Mesh(np.asarray(devices), ("core",))
    n_io = len(in_names) + len(out_names)
    sharded = jax.jit(
        shard_map(_body, mesh=mesh, in_specs=(PartitionSpec("core"),) * n_io,
                  out_specs=(PartitionSpec("core"),) * len(out_names),
                  check_rep=False),
        keep_unused=True,
    )
    return sharded, in_names, out_names, out_avals


def kernel(**inputs):
    first = "nc" not in _CACHE
    if first:
        _CACHE["nc"] = _build_module()
    nc = _CACHE["nc"]

    consts = _host_constants(inputs)
    nodes = np.asarray(inputs["nodes"], np.float32).reshape(B, N, D)
    edges = (np.asarray(inputs["edges"]) != 0).astype(np.uint8)

    in_maps = []
    for c in range(NCORES):
        m = dict(consts)
        m["nodes"] = np.ascontiguousarray(nodes[c])
        m["edges"] = edges[c]
        in_maps.append(m)

    if first:
        res = run_bass_kernel_spmd(nc, in_maps, core_ids=list(range(NCORES)))
        _CACHE["fast"] = _build_fast_path(nc)
        return np.stack([res.results[c]["out"] for c in range(NCORES)]).astype(np.float32)

    import jax
    sharded, in_names, out_names, out_avals = _CACHE["fast"]
    ckey = hash(tuple((k, v.tobytes()) for k, v in sorted(consts.items())))
    if _CACHE.get("ckey") != ckey:
        _CACHE["cdev"] = {
            n: jax.device_put(np.concatenate([np.asarray(in_maps[c][n])
                                              for c in range(NCORES)], axis=0))
            for n in in_names if n not in ("nodes", "edges")
        }
        _CACHE["zdev"] = [jax.device_put(np.zeros((NCORES * a.shape[0], *a.shape[1:]),
                                                  a.dtype)) for a in out_avals]
        _CACHE["ckey"] = ckey
    cdev = _CACHE["cdev"]
    concat_in = [cdev[n] if n in cdev else
                 np.concatenate([np.asarray(in_maps[c][n]) for c in range(NCORES)], axis=0)
                 for n in in_names]
    outs = sharded(*concat_in, *_CACHE["zdev"])
    i = out_names.index("out")
    return np.asarray(outs[i]).reshape(NCORES, N, H).astype(np.float32)
